# revision 18
# baseline (speedup 1.0000x reference)
"""BEV-pool (segment-sum scatter) Trainium2 kernel for nn_BaseDepthTransform.

Design:
  Host (numpy): replicate the reference geometry -> per-point flat BEV bin id
  (depends only on the small camera matrices, not on x). Sort points by bin.
  Greedily cut the sorted stream into "groups": up to KA*128 points spanning
  < W=16 distinct bins, each group = up to KA=8 point-tiles of 128. Binary-
  decompose group tile-counts into classes {8,4,2,1} so every class has a
  uniform static schedule. Ship, per core: an fp8(e3m4) feature stream and a
  1-byte-per-point local-bin-id stream; the [128 x 16] one-hot used by the
  matmul is built ON DEVICE by the (otherwise idle) Vector engine comparing
  lids against an iota constant.

  Device (Bass/Tile, SPMD x8): per group, chain c matmuls
  (one-hot^T @ feats) accumulating the group's [16,80] segment sums in PSUM,
  copy PSUM->SBUF on the Scalar engine, DMA to a per-group output slot from
  the GpSimd engine. No dynamic addressing, no collectives.

  Host reassembly: out[group] is added into grid[base:base+16] (groups may
  share bins across classes/cores; addition commutes).

  Feature quantization: e3m4 (4 mantissa bits) gives ~1.34e-2 global rel err
  (vs 2e-2 budget); e4m3 measures 2.66e-2 and fails. bf16 is 2x the bytes.
"""
import sys
sys.path.insert(0, '/opt/trn_rl_repo')

import numpy as np
import ml_dtypes

FP8E3 = ml_dtypes.float8_e3m4   # feats + lids + one-hot dtype (1B; ints
                                # 0..16 and 32 exact; matmul-valid)

# ---- static problem config (mirrors the reference) ----
IH, IW = 256, 704
FH, FW = 32, 88
D = 118
C = 80
NXg, NYg, NZg = 360, 360, 1
BXc = np.array([-53.85, -53.85, 0.0], np.float32)
DXc = np.array([0.3, 0.3, 20.0], np.float32)
NBINS = NZg * NXg * NYg  # 129600
W = 16                   # bins per group window
KA = 8                   # max tiles per group / PSUM chain
NCORES = 8
CLASSES = (8, 4, 2, 1)
# groups per DMA chunk, per class (roughly 2MB fp8 feats per chunk)
CHUNK_GROUPS = {8: 24, 4: 96, 2: 96, 1: 192}
PSUM_SLOTS_PER_BANK = 6  # 6 x 80 f32 = 480 of 512
WAVE = 12                # groups per PSUM wave (2 banks)
LID_PAD = 32.0           # lid value for padded rows: not in 0..15, e4m3-exact

_BUILD_CACHE = {}


def _frustum():
    ds = np.arange(1.0, 60.0, 0.5, dtype=np.float32)
    xs = np.linspace(0.0, IW - 1.0, FW, dtype=np.float32)
    ys = np.linspace(0.0, IH - 1.0, FH, dtype=np.float32)
    ds_g = np.broadcast_to(ds[:, None, None], (D, FH, FW))
    xs_g = np.broadcast_to(xs[None, None, :], (D, FH, FW))
    ys_g = np.broadcast_to(ys[None, :, None], (D, FH, FW))
    return np.stack([xs_g, ys_g, ds_g], axis=-1)  # [D,FH,FW,3]


def _get_geometry(c2l_rots, c2l_trans, intrins, post_rots, post_trans,
                  extra_rots, extra_trans):
    fr = _frustum()
    pts = fr[None, None] - post_trans[:, :, None, None, None, :]
    inv_pr = np.linalg.inv(post_rots).astype(np.float32)
    pts = np.einsum('bnij,bndhwj->bndhwi', inv_pr, pts).astype(np.float32)
    pts = np.concatenate([pts[..., :2] * pts[..., 2:3], pts[..., 2:3]], axis=-1)
    combine = np.einsum(
        'bnij,bnjk->bnik', c2l_rots, np.linalg.inv(intrins).astype(np.float32)
    ).astype(np.float32)
    pts = np.einsum('bnij,bndhwj->bndhwi', combine, pts).astype(np.float32)
    pts = pts + c2l_trans[:, :, None, None, None, :]
    pts = np.einsum('bij,bndhwj->bndhwi', extra_rots, pts).astype(np.float32)
    pts = pts + extra_trans[:, None, None, None, None, :]
    return pts  # [B,N,D,FH,FW,3]


def _flat_bins(geom):
    """Per-point flat bin id (int64), -1 for dropped points."""
    coords = ((geom - (BXc - DXc / 2.0)) / DXc).astype(np.int32)
    B = coords.shape[0]
    coords = coords.reshape(B, -1, 3)
    cx, cy, cz = coords[..., 0], coords[..., 1], coords[..., 2]
    kept = (cx >= 0) & (cx < NXg) & (cy >= 0) & (cy < NYg) & (cz >= 0) & (cz < NZg)
    flat = ((cz.astype(np.int64) * NXg + cx) * NYg + cy)
    flat = np.where(kept, flat, -1)
    return flat  # [B, Np]


def _cut_groups(fk_sorted):
    """Greedy: groups of <=KA*128 points spanning < W bins, binary-decomposed
    into class segments [(cls, start, npts, base), ...] in stream order."""
    n = len(fk_sorted)
    segs = []
    i = 0
    while i < n:
        hi = np.searchsorted(fk_sorted, fk_sorted[i] + W, side='left')
        j = min(i + KA * 128, hi, n)
        npts = j - i
        base = int(fk_sorted[i])
        nt = (npts + 127) // 128
        s = i
        for c in CLASSES:
            while nt >= c:
                ln = min(c * 128, j - s)
                segs.append((c, s, ln, base))
                s += ln
                nt -= c
        i = j
    return segs


def _split_classes(segs):
    """Per class: contiguous split across cores balanced by group count.
    Gmax = exact max per-core count (no chunk rounding; final DMA chunk and
    final PSUM wave may be partial -- the static schedule handles it)."""
    out = {}
    for c in CLASSES:
        cl = [s for s in segs if s[0] == c]
        G = len(cl)
        per = []
        for ci in range(NCORES):
            lo = (G * ci) // NCORES
            hi = (G * (ci + 1)) // NCORES
            per.append(cl[lo:hi])
        Gmax = max(1, max(len(p) for p in per))
        out[c] = (per, Gmax)
    return out


def _build_core_inputs(class_split, fk_sorted, pidx_sorted, xflat_q):
    """Build per-core input dict: per class feats + lid streams."""
    maps = [dict() for _ in range(NCORES)]
    meta = {c: [] for c in CLASSES}  # per class: percore array of bases
    for c in CLASSES:
        per, Gmax = class_split[c]
        T = Gmax * c
        CH = CHUNK_GROUPS[c]
        nch = (Gmax + CH - 1) // CH
        Tp = nch * CH * c            # tile-count padded to chunk multiple for
        TC = CH * c                  # tiles per (full) chunk
        for ci in range(NCORES):     # uniform [nch,128,TC*(C+1)] dram layout:
            segs = per[ci]           # per chunk, lids block then feats block
            feats = np.zeros((Tp, 128, C), FP8E3)
            lids = np.full((Tp, 128), LID_PAD, FP8E3)
            bases = np.full((Gmax,), -1, np.int64)
            for gi, (_, s, ln, base) in enumerate(segs):
                bases[gi] = base
                lv = (fk_sorted[s:s + ln] - base).astype(np.int64)
                pix = pidx_sorted[s:s + ln]
                t0 = gi * c
                nt = (ln + 127) // 128
                for k in range(nt):
                    a, b = k * 128, min((k + 1) * 128, ln)
                    m = b - a
                    feats[t0 + k, :m] = xflat_q[pix[a:b]]
                    lids[t0 + k, :m] = lv[a:b].astype(FP8E3)
            f = feats.reshape(nch, TC, 128, C).transpose(0, 2, 1, 3) \
                     .reshape(nch, 128, TC * C)
            l8 = lids.reshape(nch, TC, 128).transpose(0, 2, 1) \
                     .reshape(nch, 128, TC)
            maps[ci][f"feats{c}"] = np.ascontiguousarray(
                np.concatenate([l8, f], axis=2))
            meta[c].append(bases)
    iota = np.broadcast_to(np.arange(W, dtype=np.float32), (128, W))
    for ci in range(NCORES):
        maps[ci]["iota"] = np.ascontiguousarray(iota.astype(FP8E3))
    return maps, meta


def _build_bass(shape_key):
    """shape_key: tuple of (cls, Gmax) pairs."""
    if shape_key in _BUILD_CACHE:
        return _BUILD_CACHE[shape_key]
    from concourse import bass, mybir, tile, bacc

    nc = bacc.Bacc()
    params = {}
    for c, Gmax in shape_key:
        CH = CHUNK_GROUPS[c]
        nch = (Gmax + CH - 1) // CH
        params[f"feats{c}"] = nc.declare_dram_parameter(
            f"feats{c}", [nch, 128, CH * c * (C + 1)], mybir.dt.float8e3,
            isOutput=False)
        params[f"out{c}"] = nc.declare_dram_parameter(
            f"out{c}", [W, Gmax, C], mybir.dt.bfloat16, isOutput=True)
    params["iota"] = nc.declare_dram_parameter(
        "iota", [128, W], mybir.dt.float8e3, isOutput=False)

    def slot_off(s):
        return (s // PSUM_SLOTS_PER_BANK) * 512 + (s % PSUM_SLOTS_PER_BANK) * C

    # small classes first: PE starts on a cheap chunk while the class-8
    # stream is still in flight
    chunk_order = []
    for c, Gmax in shape_key:
        CH = CHUNK_GROUPS[c]
        nch = (Gmax + CH - 1) // CH
        for ch in range(nch):
            chunk_order.append((c, Gmax, ch))
    chunk_order.sort(key=lambda t: (t[0] if t[0] != 8 else 9, t[2]))

    with tile.TileContext(nc) as tc:
        with tc.tile_pool(name="fstream", bufs=5) as fpool, \
             tc.tile_pool(name="stage", bufs=6) as spool, \
             tc.tile_pool(name="const", bufs=1) as cpool, \
             tc.tile_pool(name="psum", bufs=4, space="PSUM") as psum_pool:
            iota_t = cpool.tile([128, W], mybir.dt.float8e3, tag="iota")
            nc.sync.dma_start(iota_t[:], params["iota"][:, :])
            for c, Gmax, ch in chunk_order:
                CH = CHUNK_GROUPS[c]
                CHg = min(CH, Gmax - ch * CH)   # groups in this chunk
                T = CHg * c                     # live tiles in this chunk
                TC = CH * c
                # chunk dram layout: [lids (TC bytes) | feats (TC*C bytes)];
                # lids land in a separate tile via a tiny first DMA so the
                # one-hot build starts while the feats DMA is in flight
                lchunk = fpool.tile([128, TC], mybir.dt.float8e3,
                                    tag="lchunk")
                nc.sync.dma_start(lchunk[:], params[f"feats{c}"][ch, :, :TC])
                fchunk = fpool.tile([128, TC * C], mybir.dt.float8e3,
                                    tag="fchunk")
                nc.sync.dma_start(fchunk[:, :T * C],
                                  params[f"feats{c}"][ch, :, TC:TC + T * C])
                feat_off = 0
                ochunk = fpool.tile([128, TC * W], mybir.dt.float8e3,
                                    tag="ochunk")
                # one-hot build: oh[p, t*W+j] = (lids[p,t] == iota[p,j])
                dst = bass.AP(ochunk[:].tensor, ochunk[:].offset,
                              [ochunk[:].ap[0], [W, T], [1, W]])
                src0 = bass.AP(lchunk[:].tensor, lchunk[:].offset,
                               [lchunk[:].ap[0], [1, T], [0, W]])
                src1 = bass.AP(iota_t[:].tensor, iota_t[:].offset,
                               [iota_t[:].ap[0], [0, T], [1, W]])
                nc.vector.tensor_tensor(dst, src0, src1,
                                        mybir.AluOpType.is_equal)
                nwave = (CHg + WAVE - 1) // WAVE
                for wv in range(nwave):
                    g0 = wv * WAVE
                    NW = min(WAVE, CHg - g0)
                    mega = psum_pool.tile([W, 2 * 512], mybir.dt.float32,
                                          tag="ps")

                    def mm(sp, k):
                        ta = (g0 + sp) * c + k
                        nc.tensor.matmul(
                            out=mega[:, slot_off(sp):slot_off(sp) + C],
                            lhsT=ochunk[:, ta * W:(ta + 1) * W],
                            rhs=fchunk[:, feat_off + ta * C:
                                       feat_off + (ta + 1) * C],
                            start=(k == 0), stop=(k == c - 1))

                    # chains interleave only ACROSS psum banks (same-bank
                    # interleaved accumulation corrupts); leftovers run solo
                    npair = max(0, NW - PSUM_SLOTS_PER_BANK)
                    for gp in range(npair):
                        for k in range(c):
                            mm(gp, k)
                            mm(gp + PSUM_SLOTS_PER_BANK, k)
                    for sp in range(npair, min(PSUM_SLOTS_PER_BANK, NW)):
                        for k in range(c):
                            mm(sp, k)
                    st = spool.tile([W, WAVE * C], mybir.dt.bfloat16, tag="st")
                    # copy PSUM slots bank by bank (last bank may be partial)
                    s0 = 0
                    while s0 < NW:
                        cnt = min(PSUM_SLOTS_PER_BANK, NW - s0)
                        nc.scalar.copy(
                            st[:, s0 * C:(s0 + cnt) * C],
                            mega[:, (s0 // PSUM_SLOTS_PER_BANK) * 512:
                                    (s0 // PSUM_SLOTS_PER_BANK) * 512 + cnt * C])
                        s0 += cnt
                    nc.gpsimd.dma_start(
                        params[f"out{c}"][:, ch * CH + g0:ch * CH + g0 + NW, :],
                        st[:, :NW * C])
    nc.finalize()
    _BUILD_CACHE[shape_key] = nc
    return nc


def run_scheduled(x, flat, trace=False, trace_cores=None):
    """Core pipeline given precomputed flat bins; returns (grid, results)."""
    from concourse.bass_utils import run_bass_kernel_spmd

    xflat_q = np.ascontiguousarray(x.reshape(-1, C)).astype(FP8E3)
    kept_idx = np.nonzero(flat >= 0)[0]
    fk = flat[kept_idx]
    order = np.argsort(fk, kind='stable')
    fk_sorted = fk[order]
    pidx_sorted = kept_idx[order]

    segs = _cut_groups(fk_sorted)
    class_split = _split_classes(segs)
    shape_key = tuple((c, class_split[c][1]) for c in CLASSES)

    maps, meta = _build_core_inputs(class_split, fk_sorted, pidx_sorted,
                                    xflat_q)
    nc = _build_bass(shape_key)
    res = run_bass_kernel_spmd(nc, maps, core_ids=list(range(NCORES)),
                               trace=trace, trace_cores=trace_cores)

    grid = np.zeros((NBINS + W, C), np.float32)
    for c in CLASSES:
        for ci in range(NCORES):
            outs = np.asarray(res.results[ci][f"out{c}"],
                              np.float32)          # [W, Gmax, C]
            bases = meta[c][ci]
            for gi in range(len(bases)):
                base = bases[gi]
                if base >= 0:
                    grid[base:base + W] += outs[:, gi]
    return grid[:NBINS], res


def kernel(x, camera2lidar_rots, camera2lidar_trans, intrins, post_rots,
           post_trans, extra_rots, extra_trans):
    x = np.asarray(x, np.float32)
    B, N = x.shape[0], x.shape[1]
    assert (B, N) == (1, 6) and x.shape[2:] == (D, FH, FW, C), x.shape

    geom = _get_geometry(
        np.asarray(camera2lidar_rots, np.float32),
        np.asarray(camera2lidar_trans, np.float32),
        np.asarray(intrins, np.float32),
        np.asarray(post_rots, np.float32),
        np.asarray(post_trans, np.float32),
        np.asarray(extra_rots, np.float32),
        np.asarray(extra_trans, np.float32),
    )
    flat = _flat_bins(geom)[0]          # [Np]
    grid, _ = run_scheduled(x, flat)
    outp = grid.reshape(NXg, NYg, C).transpose(2, 0, 1)[None]  # [1,C,NX,NY]
    return np.ascontiguousarray(outp)
